# revision 19
# baseline (speedup 1.0000x reference)
"""Trainium2 Bass kernel: AtomSelfInteraction GNN edge update.

out = silu(concat([h[idx_i], h[idx_j], m_ij], -1) @ W)

Strategy (8 NeuronCores, SPMD data-parallel over edges):
  - Each core owns E/8 = 25000 edges.
  - dma_gather is the fast path for the h-row gathers but sign-extends its
    int16 indices on HW, so h is split into table A (rows < 32768) and
    table B (the rest). The host partitions each core's edges into 4 groups
    by (which table idx_i needs, which table idx_j needs), pads each group
    to whole 128-edge tiles (group capacities = max over cores, baked into
    the compiled kernel), and un-permutes device output rows at the end.
  - Host prep: h, W, m_ij cast to bf16; m_ij pre-transposed per core to
    [512, e_pad] so its K-dim lands on SBUF partitions; gather indices
    pre-wrapped into the dma_gather int16 layout (16-partition wrap,
    replicated across the 8 Q7 cores).
  - Device, per slab of <=7 tiles (<=896 edges; dma_gather caps at 1024
    descriptors/call): 2 dma_gathers (side i, side j) spread over 4 SWDGE
    queues; per 128-edge tile: 4 PE transposes (via identity) put gathered
    h on K-partitions, 8 bf16 matmuls (K=1024 in 128-chunks) accumulate
    into a PSUM bank (m-part first so they overlap the DVE PSUM->SBUF copy
    of the transposed h), SiLU on ScalarE, f32 DMA out.
"""

import numpy as np
import ml_dtypes

import concourse.bass as bass
import concourse.tile as tile
from concourse import bacc
from concourse import mybir
from concourse.bass_utils import run_bass_kernel_spmd
from concourse.masks import make_identity

P = 128
N_CORES = 8
N_NODES = 50000
SPLIT = 32768                      # int16-addressable rows in table A
E_TOTAL = 200000
EMB_ATOM = 256
EMB_EDGE = 512
IN_SIZE = 2 * EMB_ATOM + EMB_EDGE  # 1024

TILES_PER_SLAB = 7                 # 896 edges: dma_gather caps at 1024 desc/call
NUM_SWDGE_QUEUES = 4

BF16 = mybir.dt.bfloat16
F32 = mybir.dt.float32
I16 = mybir.dt.int16

K_CHUNKS = IN_SIZE // P            # 8
M_CHUNKS = EMB_EDGE // P           # 4 (m_ij feature chunks, K chunks 4..7)
H_CHUNKS = EMB_ATOM // P           # 2 per h side

# group id -> (side-i uses table B, side-j uses table B)
GROUPS = ((False, False), (False, True), (True, False), (True, True))


def build_nc(
    group_tiles,
    n_nodes=N_NODES,
    split=SPLIT,
    tiles_per_slab=TILES_PER_SLAB,
    act=mybir.ActivationFunctionType.Silu,
):
    total_tiles = sum(group_tiles)
    e_pad = total_tiles * P
    idx_cols = total_tiles * P // 16

    nc = bacc.Bacc(
        "TRN2",
        target_bir_lowering=False,
        debug=False,
        num_swdge_queues=NUM_SWDGE_QUEUES,
    )
    ha_d = nc.dram_tensor("h_a", [split, EMB_ATOM], BF16, kind="ExternalInput").ap()
    hb_d = nc.dram_tensor(
        "h_b", [max(n_nodes - split, 1), EMB_ATOM], BF16, kind="ExternalInput"
    ).ap()
    mt_d = nc.dram_tensor("m_t", [EMB_EDGE, e_pad], BF16, kind="ExternalInput").ap()
    ii_d = nc.dram_tensor("idx_i", [P, idx_cols], I16, kind="ExternalInput").ap()
    ij_d = nc.dram_tensor("idx_j", [P, idx_cols], I16, kind="ExternalInput").ap()
    w_d = nc.dram_tensor("w_bf", [IN_SIZE, EMB_EDGE], BF16, kind="ExternalInput").ap()
    out_d = nc.dram_tensor("out", [e_pad, EMB_EDGE], F32, kind="ExternalOutput").ap()

    with tile.TileContext(nc) as tc:
        with (
            tc.tile_pool(name="const", bufs=1) as const_pool,
            tc.tile_pool(name="mt", bufs=2) as mt_pool,
            tc.tile_pool(name="hg", bufs=3) as hg_pool,
            tc.tile_pool(name="xt", bufs=4) as xt_pool,
            tc.tile_pool(name="pst", bufs=6, space="PSUM") as pst_pool,
            tc.tile_pool(name="acc", bufs=2, space="PSUM") as acc_pool,
            tc.tile_pool(name="outp", bufs=4) as out_pool,
        ):
            w_tile = const_pool.tile([P, K_CHUNKS, EMB_EDGE], BF16)
            nc.sync.dma_start(w_tile[:], w_d.rearrange("(k p) o -> p k o", p=P))
            ident = const_pool.tile([P, P], BF16)
            make_identity(nc, ident[:])
            idxi_t = const_pool.tile([P, idx_cols], I16, tag="idxi")
            nc.sync.dma_start(idxi_t[:], ii_d[:])
            idxj_t = const_pool.tile([P, idx_cols], I16, tag="idxj")
            nc.sync.dma_start(idxj_t[:], ij_d[:])

            mt_r = mt_d.rearrange("(c p) e -> p c e", p=P)  # [128, 4, e_pad]

            # First xt-matmul of past tiles; scheduling-only deps keep the PE
            # stream interleaved so transposes into reused PSUM slots don't
            # need a 2nd sync wait (walrus MM struct fits one).
            xt_mm_hist = []
            tile_base = 0      # global tile counter
            q = 0              # SWDGE queue round-robin

            for g, (i_in_b, j_in_b) in enumerate(GROUPS):
                tabs = {"i": hb_d if i_in_b else ha_d, "j": hb_d if j_in_b else ha_d}
                gt = group_tiles[g]
                for s0 in range(0, gt, tiles_per_slab):
                    nt = min(tiles_per_slab, gt - s0)
                    t0 = tile_base + s0
                    e0 = t0 * P
                    es = nt * P
                    mt_slab = mt_pool.tile([P, M_CHUNKS, es], BF16, tag="mt")
                    nc.sync.dma_start(mt_slab[:], mt_r[:, :, e0 : e0 + es])

                    sel = {}
                    for side in ("i", "j"):
                        idx_t = idxi_t if side == "i" else idxj_t
                        gg = hg_pool.tile([P, nt, EMB_ATOM], BF16, tag=f"g{side}")
                        nc.gpsimd.dma_gather(
                            out_ap=gg[:],
                            in_ap=tabs[side],
                            idxs_ap=idx_t[:, e0 // 16 : (e0 + es) // 16],
                            num_idxs=es,
                            num_idxs_reg=es,
                            elem_size=EMB_ATOM,
                            queue_num=q % NUM_SWDGE_QUEUES,
                        )
                        q += 1
                        sel[side] = gg

                    for t in range(nt):
                        # Transpose gathered h rows onto K-partitions (PE).
                        pst = pst_pool.tile([P, 2 * EMB_ATOM], BF16, tag="pst")
                        tr0 = None
                        for c in range(H_CHUNKS):
                            tr = nc.tensor.transpose(
                                pst[:, c * P : (c + 1) * P],
                                sel["i"][:, t, c * P : (c + 1) * P],
                                ident[:],
                            )
                            tr0 = tr0 or tr
                        if len(xt_mm_hist) >= 2:
                            bass._add_dep_helper(
                                tr0.ins,
                                xt_mm_hist[-2].ins,
                                sync=False,
                                reason="PE interleave: keep DVE tick observed",
                            )
                        for c in range(H_CHUNKS):
                            nc.tensor.transpose(
                                pst[:, (H_CHUNKS + c) * P : (H_CHUNKS + c + 1) * P],
                                sel["j"][:, t, c * P : (c + 1) * P],
                                ident[:],
                            )
                        xt = xt_pool.tile([P, 2 * EMB_ATOM], BF16)
                        nc.vector.tensor_copy(xt[:], pst[:])

                        acc = acc_pool.tile([P, EMB_EDGE], F32)
                        # m-part matmuls first: they only need mt_slab, so the
                        # PE keeps streaming while the DVE copy of xt lands.
                        for c in range(M_CHUNKS):
                            nc.tensor.matmul(
                                acc[:],
                                lhsT=mt_slab[:, c, t * P : (t + 1) * P],
                                rhs=w_tile[:, 2 * H_CHUNKS + c, :],
                                start=(c == 0),
                                stop=False,
                            )
                        for k in range(2 * H_CHUNKS):
                            mm = nc.tensor.matmul(
                                acc[:],
                                lhsT=xt[:, k * P : (k + 1) * P],
                                rhs=w_tile[:, k, :],
                                start=False,
                                stop=(k == 2 * H_CHUNKS - 1),
                            )
                            if k == 0:
                                xt_mm_hist.append(mm)

                        ot = out_pool.tile([P, EMB_EDGE], F32)
                        nc.scalar.activation(ot[:], acc[:], act)
                        e_t = e0 + t * P
                        nc.sync.dma_start(out_d[e_t : e_t + P, :], ot[:])
                tile_base += gt
    nc.compile()
    return nc


def _wrap_idx16(vals):
    """[n] int array (n % 128 == 0) -> [128, n//16] int16 in dma_gather
    layout: list element k sits at partition k%16, column k//16, replicated
    across the 8 groups of 16 partitions."""
    n = vals.size
    blk = vals.reshape(n // 16, 16).T.astype(np.int16)  # [16, n/16]
    return np.ascontiguousarray(np.tile(blk, (8, 1)))


def partition_core(ix_i, ix_j, split):
    """Group edges by (table_i, table_j). Returns (order, counts):
    order = edge indices sorted by group (stable), counts per group."""
    gid = (ix_i >= split).astype(np.int8) * 2 + (ix_j >= split)
    order = np.argsort(gid, kind="stable")
    counts = np.bincount(gid, minlength=4)
    return order, counts


def prep_core_inputs(h_a, h_b, w_bf, m, ix_i, ix_j, order, counts, group_tiles,
                     split):
    """Build one core's padded, grouped input map."""
    total_tiles = sum(group_tiles)
    e_pad = total_tiles * P
    m_pad = np.zeros((e_pad, EMB_EDGE), np.float32)
    ii_pad = np.zeros(e_pad, np.int64)
    jj_pad = np.zeros(e_pad, np.int64)
    pos = 0
    off = 0
    for g in range(4):
        n = int(counts[g])
        sel = order[pos : pos + n]
        m_pad[off : off + n] = m[sel]
        ii_pad[off : off + n] = ix_i[sel]
        jj_pad[off : off + n] = ix_j[sel]
        # padding rows keep idx 0, which is valid for either table
        pos += n
        off += group_tiles[g] * P
    i_in_b = np.repeat([b for b, _ in GROUPS], np.array(group_tiles) * P)
    j_in_b = np.repeat([b for _, b in GROUPS], np.array(group_tiles) * P)
    ii_dev = np.where(i_in_b, np.maximum(ii_pad - split, 0), ii_pad)
    jj_dev = np.where(j_in_b, np.maximum(jj_pad - split, 0), jj_pad)
    return {
        "h_a": h_a,
        "h_b": h_b,
        "m_t": np.ascontiguousarray(m_pad.T).astype(ml_dtypes.bfloat16),
        "idx_i": _wrap_idx16(ii_dev),
        "idx_j": _wrap_idx16(jj_dev),
        "w_bf": w_bf,
    }


def _ensure_ntff_hook():
    """Make trace=True work: register the ctypes NTFF profile hook when the
    image's antenv package lacks axon_hooks (boot degrades silently)."""
    import sys
    import types

    try:
        from antenv.axon_hooks import get_axon_ntff_profile_hook  # noqa: F401

        return
    except ImportError:
        pass
    import antenv
    from trn_agent_boot.trn_boot import _ntff_profile_via_ctypes

    hook = _ntff_profile_via_ctypes("/opt/axon/libaxon_pjrt.so")
    mod = types.ModuleType("antenv.axon_hooks")
    mod.get_axon_ntff_profile_hook = lambda: hook
    mod.set_axon_ntff_profile_hook = lambda h: None
    sys.modules["antenv.axon_hooks"] = mod
    antenv.axon_hooks = mod


_NC_CACHE = {}


def kernel(h, m_ij, idx_i, idx_j, W, trace=False, split=SPLIT):
    e_total = m_ij.shape[0]
    e_core = e_total // N_CORES
    if trace:
        _ensure_ntff_hook()

    h_bf = np.asarray(h).astype(ml_dtypes.bfloat16)
    h_a, h_b = h_bf[:split], h_bf[split:]
    if h_b.size == 0:
        h_b = np.zeros((1, EMB_ATOM), ml_dtypes.bfloat16)
    w_bf = np.asarray(W).astype(ml_dtypes.bfloat16)
    idx_i = np.asarray(idx_i)
    idx_j = np.asarray(idx_j)

    parts = []
    for c in range(N_CORES):
        sl = slice(c * e_core, (c + 1) * e_core)
        parts.append(partition_core(idx_i[sl], idx_j[sl], split))
    group_tiles = tuple(
        int(max((p[1][g] + P - 1) // P for p in parts)) for g in range(4)
    )

    key = (group_tiles, split, h.shape[0])
    if key not in _NC_CACHE:
        _NC_CACHE[key] = build_nc(group_tiles, n_nodes=h.shape[0], split=split)
    nc = _NC_CACHE[key]

    in_maps = []
    for c in range(N_CORES):
        sl = slice(c * e_core, (c + 1) * e_core)
        order, counts = parts[c]
        in_maps.append(
            prep_core_inputs(
                h_a, h_b, w_bf, m_ij[sl], idx_i[sl], idx_j[sl],
                order, counts, group_tiles, split,
            )
        )

    res = run_bass_kernel_spmd(nc, in_maps, core_ids=list(range(N_CORES)), trace=trace)

    out = np.empty((e_total, EMB_EDGE), np.float32)
    for c in range(N_CORES):
        order, counts = parts[c]
        dev = res.results[c]["out"]
        pos = 0
        off = 0
        core_out = out[c * e_core : (c + 1) * e_core]
        for g in range(4):
            n = int(counts[g])
            core_out[order[pos : pos + n]] = dev[off : off + n]
            pos += n
            off += group_tiles[g] * P
    if trace:
        kernel.last_result = res
    return out


# revision 21
# speedup vs baseline: 1.1482x; 1.1482x over previous
"""Trainium2 Bass kernel: AtomSelfInteraction GNN edge update.

out = silu(concat([h[idx_i], h[idx_j], m_ij], -1) @ W)

Strategy (8 NeuronCores, SPMD data-parallel over edges):
  - Each core owns E/8 = 25000 edges.
  - h-row gathers use dma_gather(transpose=True): gathered rows arrive in
    SBUF already transposed ([128 feature partitions, 2 chunks, edges]) --
    exactly the stationary-operand layout the matmuls need, so no on-chip
    transposes at all. dma_gather sign-extends its int16 indices on HW, so
    h is split into table A (rows < 32768) and table B (the rest); the host
    partitions each core's edges into 4 groups by (table_i, table_j), pads
    each group to whole 128-edge tiles (group capacities = max over cores,
    baked into the compiled kernel), and un-permutes output rows at the end.
  - Host prep: h, W, m_ij cast to bf16; m_ij pre-transposed per core to
    [512, e_pad] so its K-dim lands on SBUF partitions; gather indices
    pre-wrapped into the dma_gather int16 layout (16-partition wrap,
    replicated across the 8 Q7 cores).
  - Device, per slab of <=8 tiles (<=1024 edges; dma_gather caps at 1024
    descriptors/call): 2 transposed dma_gathers (side i, side j) spread
    over 4 SWDGE queues; per 128-edge tile: 8 bf16 matmuls (K=1024 in
    128-chunks) accumulate into a PSUM bank, SiLU on ScalarE (PSUM ->
    SBUF bf16), bf16 DMA out (host upcasts to f32).
"""

import numpy as np
import ml_dtypes

import concourse.bass as bass
import concourse.tile as tile
from concourse import bacc
from concourse import mybir
from concourse.bass_utils import run_bass_kernel_spmd

P = 128
N_CORES = 8
N_NODES = 50000
SPLIT = 32768                      # int16-addressable rows in table A
E_TOTAL = 200000
EMB_ATOM = 256
EMB_EDGE = 512
IN_SIZE = 2 * EMB_ATOM + EMB_EDGE  # 1024

TILES_PER_SLAB = 7                 # 896 edges: transposed dma_gather rejects 1024
NUM_SWDGE_QUEUES = 4

BF16 = mybir.dt.bfloat16
F32 = mybir.dt.float32
I16 = mybir.dt.int16

K_CHUNKS = IN_SIZE // P            # 8
M_CHUNKS = EMB_EDGE // P           # 4 (m_ij feature chunks, K chunks 4..7)
H_CHUNKS = EMB_ATOM // P           # 2 per h side

# group id -> (side-i uses table B, side-j uses table B)
GROUPS = ((False, False), (False, True), (True, False), (True, True))


def build_nc(
    group_tiles,
    n_nodes=N_NODES,
    split=SPLIT,
    tiles_per_slab=TILES_PER_SLAB,
    act=mybir.ActivationFunctionType.Silu,
    out_dtype=BF16,
):
    total_tiles = sum(group_tiles)
    e_pad = total_tiles * P
    idx_cols = total_tiles * P // 16

    nc = bacc.Bacc(
        "TRN2",
        target_bir_lowering=False,
        debug=False,
        num_swdge_queues=NUM_SWDGE_QUEUES,
    )
    ha_d = nc.dram_tensor("h_a", [split, EMB_ATOM], BF16, kind="ExternalInput").ap()
    hb_d = nc.dram_tensor(
        "h_b", [max(n_nodes - split, 1), EMB_ATOM], BF16, kind="ExternalInput"
    ).ap()
    mt_d = nc.dram_tensor("m_t", [EMB_EDGE, e_pad], BF16, kind="ExternalInput").ap()
    ii_d = nc.dram_tensor("idx_i", [P, idx_cols], I16, kind="ExternalInput").ap()
    ij_d = nc.dram_tensor("idx_j", [P, idx_cols], I16, kind="ExternalInput").ap()
    w_d = nc.dram_tensor("w_bf", [IN_SIZE, EMB_EDGE], BF16, kind="ExternalInput").ap()
    out_d = nc.dram_tensor(
        "out", [e_pad, EMB_EDGE], out_dtype, kind="ExternalOutput"
    ).ap()

    with tile.TileContext(nc) as tc:
        with (
            tc.tile_pool(name="const", bufs=1) as const_pool,
            tc.tile_pool(name="mt", bufs=3) as mt_pool,
            tc.tile_pool(name="hg", bufs=3) as hg_pool,
            tc.tile_pool(name="acc", bufs=6, space="PSUM") as acc_pool,
            tc.tile_pool(name="outp", bufs=6) as out_pool,
        ):
            w_tile = const_pool.tile([P, K_CHUNKS, EMB_EDGE], BF16)
            nc.sync.dma_start(w_tile[:], w_d.rearrange("(k p) o -> p k o", p=P))
            idxi_t = const_pool.tile([P, idx_cols], I16, tag="idxi")
            nc.sync.dma_start(idxi_t[:], ii_d[:])
            idxj_t = const_pool.tile([P, idx_cols], I16, tag="idxj")
            nc.sync.dma_start(idxj_t[:], ij_d[:])

            mt_r = mt_d.rearrange("(c p) e -> p c e", p=P)  # [128, 4, e_pad]

            tile_base = 0      # global tile counter
            q = 0              # SWDGE queue round-robin

            for g, (i_in_b, j_in_b) in enumerate(GROUPS):
                tabs = {"i": hb_d if i_in_b else ha_d, "j": hb_d if j_in_b else ha_d}
                gt = group_tiles[g]
                for s0 in range(0, gt, tiles_per_slab):
                    nt = min(tiles_per_slab, gt - s0)
                    t0 = tile_base + s0
                    e0 = t0 * P
                    es = nt * P
                    mt_slab = mt_pool.tile([P, M_CHUNKS, es], BF16, tag="mt")
                    nc.sync.dma_start(mt_slab[:], mt_r[:, :, e0 : e0 + es])

                    gat = {}
                    for side in ("i", "j"):
                        idx_t = idxi_t if side == "i" else idxj_t
                        gg = hg_pool.tile([P, H_CHUNKS, es], BF16, tag=f"g{side}")
                        nc.gpsimd.dma_gather(
                            out_ap=gg[:],
                            in_ap=tabs[side],
                            idxs_ap=idx_t[:, e0 // 16 : (e0 + es) // 16],
                            num_idxs=es,
                            num_idxs_reg=es,
                            elem_size=EMB_ATOM,
                            transpose=True,
                            queue_num=q % NUM_SWDGE_QUEUES,
                        )
                        q += 1
                        gat[side] = gg

                    for t in range(nt):
                        acc = acc_pool.tile([P, EMB_EDGE], F32)
                        esl = slice(t * P, (t + 1) * P)
                        for c in range(M_CHUNKS):
                            nc.tensor.matmul(
                                acc[:],
                                lhsT=mt_slab[:, c, esl],
                                rhs=w_tile[:, 2 * H_CHUNKS + c, :],
                                start=(c == 0),
                                stop=False,
                            )
                        for c in range(H_CHUNKS):
                            nc.tensor.matmul(
                                acc[:],
                                lhsT=gat["i"][:, c, esl],
                                rhs=w_tile[:, c, :],
                                start=False,
                                stop=False,
                            )
                        for c in range(H_CHUNKS):
                            nc.tensor.matmul(
                                acc[:],
                                lhsT=gat["j"][:, c, esl],
                                rhs=w_tile[:, H_CHUNKS + c, :],
                                start=False,
                                stop=(c == H_CHUNKS - 1),
                            )

                        ot = out_pool.tile([P, EMB_EDGE], out_dtype)
                        nc.scalar.activation(ot[:], acc[:], act)
                        e_t = e0 + t * P
                        nc.sync.dma_start(out_d[e_t : e_t + P, :], ot[:])
                tile_base += gt
    nc.compile()
    return nc


def _wrap_idx16(vals):
    """[n] int array (n % 128 == 0) -> [128, n//16] int16 in dma_gather
    layout: list element k sits at partition k%16, column k//16, replicated
    across the 8 groups of 16 partitions."""
    n = vals.size
    blk = vals.reshape(n // 16, 16).T.astype(np.int16)  # [16, n/16]
    return np.ascontiguousarray(np.tile(blk, (8, 1)))


def partition_core(ix_i, ix_j, split):
    """Group edges by (table_i, table_j). Returns (order, counts):
    order = edge indices sorted by group (stable), counts per group."""
    gid = (ix_i >= split).astype(np.int8) * 2 + (ix_j >= split)
    order = np.argsort(gid, kind="stable")
    counts = np.bincount(gid, minlength=4)
    return order, counts


def prep_core_inputs(h_a, h_b, w_bf, m, ix_i, ix_j, order, counts, group_tiles,
                     split):
    """Build one core's padded, grouped input map."""
    total_tiles = sum(group_tiles)
    e_pad = total_tiles * P
    m_pad = np.zeros((e_pad, EMB_EDGE), np.float32)
    ii_pad = np.zeros(e_pad, np.int64)
    jj_pad = np.zeros(e_pad, np.int64)
    pos = 0
    off = 0
    for g in range(4):
        n = int(counts[g])
        sel = order[pos : pos + n]
        m_pad[off : off + n] = m[sel]
        ii_pad[off : off + n] = ix_i[sel]
        jj_pad[off : off + n] = ix_j[sel]
        # padding rows keep idx 0, which is valid for either table
        pos += n
        off += group_tiles[g] * P
    i_in_b = np.repeat([b for b, _ in GROUPS], np.array(group_tiles) * P)
    j_in_b = np.repeat([b for _, b in GROUPS], np.array(group_tiles) * P)
    ii_dev = np.where(i_in_b, np.maximum(ii_pad - split, 0), ii_pad)
    jj_dev = np.where(j_in_b, np.maximum(jj_pad - split, 0), jj_pad)
    return {
        "h_a": h_a,
        "h_b": h_b,
        "m_t": np.ascontiguousarray(m_pad.T).astype(ml_dtypes.bfloat16),
        "idx_i": _wrap_idx16(ii_dev),
        "idx_j": _wrap_idx16(jj_dev),
        "w_bf": w_bf,
    }


def _ensure_ntff_hook():
    """Make trace=True work: register the ctypes NTFF profile hook when the
    image's antenv package lacks axon_hooks (boot degrades silently)."""
    import sys
    import types

    try:
        from antenv.axon_hooks import get_axon_ntff_profile_hook  # noqa: F401

        return
    except ImportError:
        pass
    import antenv
    from trn_agent_boot.trn_boot import _ntff_profile_via_ctypes

    hook = _ntff_profile_via_ctypes("/opt/axon/libaxon_pjrt.so")
    mod = types.ModuleType("antenv.axon_hooks")
    mod.get_axon_ntff_profile_hook = lambda: hook
    mod.set_axon_ntff_profile_hook = lambda h: None
    sys.modules["antenv.axon_hooks"] = mod
    antenv.axon_hooks = mod


_NC_CACHE = {}


def kernel(h, m_ij, idx_i, idx_j, W, trace=False, split=SPLIT):
    e_total = m_ij.shape[0]
    e_core = e_total // N_CORES
    if trace:
        _ensure_ntff_hook()

    h_bf = np.asarray(h).astype(ml_dtypes.bfloat16)
    h_a, h_b = h_bf[:split], h_bf[split:]
    if h_b.size == 0:
        h_b = np.zeros((1, EMB_ATOM), ml_dtypes.bfloat16)
    w_bf = np.asarray(W).astype(ml_dtypes.bfloat16)
    idx_i = np.asarray(idx_i)
    idx_j = np.asarray(idx_j)

    parts = []
    for c in range(N_CORES):
        sl = slice(c * e_core, (c + 1) * e_core)
        parts.append(partition_core(idx_i[sl], idx_j[sl], split))
    group_tiles = tuple(
        int(max((p[1][g] + P - 1) // P for p in parts)) for g in range(4)
    )

    key = (group_tiles, split, h.shape[0])
    if key not in _NC_CACHE:
        _NC_CACHE[key] = build_nc(group_tiles, n_nodes=h.shape[0], split=split)
    nc = _NC_CACHE[key]

    in_maps = []
    for c in range(N_CORES):
        sl = slice(c * e_core, (c + 1) * e_core)
        order, counts = parts[c]
        in_maps.append(
            prep_core_inputs(
                h_a, h_b, w_bf, m_ij[sl], idx_i[sl], idx_j[sl],
                order, counts, group_tiles, split,
            )
        )

    res = run_bass_kernel_spmd(nc, in_maps, core_ids=list(range(N_CORES)), trace=trace)

    out = np.empty((e_total, EMB_EDGE), np.float32)
    for c in range(N_CORES):
        order, counts = parts[c]
        dev = res.results[c]["out"]
        pos = 0
        off = 0
        core_out = out[c * e_core : (c + 1) * e_core]
        for g in range(4):
            n = int(counts[g])
            core_out[order[pos : pos + n]] = dev[off : off + n].astype(np.float32)
            pos += n
            off += group_tiles[g] * P
    if trace:
        kernel.last_result = res
    return out


# revision 27
# speedup vs baseline: 1.1617x; 1.0118x over previous
"""Trainium2 Bass kernel: AtomSelfInteraction GNN edge update.

out = silu(concat([h[idx_i], h[idx_j], m_ij], -1) @ W)

Strategy (8 NeuronCores, SPMD data-parallel over edges):
  - Each core owns E/8 = 25000 edges.
  - h-row gathers use dma_gather(transpose=True): gathered rows arrive in
    SBUF already transposed ([128 feature partitions, 2 chunks, edges]) --
    exactly the stationary-operand layout the matmuls need, so no on-chip
    transposes at all. dma_gather sign-extends its int16 indices on HW, so
    h is split into table A (rows < 32768) and table B (the rest); the host
    partitions each core's edges into 4 groups by (table_i, table_j), pads
    each group to whole 128-edge tiles (group capacities = max over cores,
    baked into the compiled kernel), and un-permutes output rows at the end.
  - Host prep: h, W, m_ij cast to bf16; m_ij pre-transposed per core to
    [512, e_pad] so its K-dim lands on SBUF partitions; gather indices
    pre-wrapped into the dma_gather int16 layout (16-partition wrap,
    replicated across the 8 Q7 cores).
  - Device, per slab of <=8 tiles (<=1024 edges; dma_gather caps at 1024
    descriptors/call): 2 transposed dma_gathers (side i, side j) spread
    over 4 SWDGE queues; per 128-edge tile: 8 bf16 matmuls (K=1024 in
    128-chunks) accumulate into a PSUM bank, SiLU on ScalarE (PSUM ->
    SBUF bf16), bf16 DMA out (host upcasts to f32).
"""

import numpy as np
import ml_dtypes

import concourse.bass as bass
import concourse.tile as tile
from concourse import bacc
from concourse import mybir
from concourse.bass_utils import run_bass_kernel_spmd

P = 128
N_CORES = 8
N_NODES = 50000
SPLIT = 32768                      # int16-addressable rows in table A
E_TOTAL = 200000
EMB_ATOM = 256
EMB_EDGE = 512
IN_SIZE = 2 * EMB_ATOM + EMB_EDGE  # 1024

TILES_PER_SLAB = 7                 # 896 edges: transposed dma_gather rejects 1024
NUM_SWDGE_QUEUES = 4

BF16 = mybir.dt.bfloat16
F32 = mybir.dt.float32
I16 = mybir.dt.int16

K_CHUNKS = IN_SIZE // P            # 8
M_CHUNKS = EMB_EDGE // P           # 4 (m_ij feature chunks, K chunks 4..7)
H_CHUNKS = EMB_ATOM // P           # 2 per h side

# group id -> (side-i uses table B, side-j uses table B)
GROUPS = ((False, False), (False, True), (True, False), (True, True))


def build_nc(
    group_tiles,
    n_nodes=N_NODES,
    split=SPLIT,
    tiles_per_slab=TILES_PER_SLAB,
    act=mybir.ActivationFunctionType.Silu,
    out_dtype=BF16,
):
    total_tiles = sum(group_tiles)
    e_pad = total_tiles * P
    idx_cols = total_tiles * P // 16

    nc = bacc.Bacc(
        "TRN2",
        target_bir_lowering=False,
        debug=False,
        num_swdge_queues=NUM_SWDGE_QUEUES,
    )
    ha_d = nc.dram_tensor("h_a", [split, EMB_ATOM], BF16, kind="ExternalInput").ap()
    hb_d = nc.dram_tensor(
        "h_b", [max(n_nodes - split, 1), EMB_ATOM], BF16, kind="ExternalInput"
    ).ap()
    mt_d = nc.dram_tensor("m_t", [EMB_EDGE, e_pad], BF16, kind="ExternalInput").ap()
    ii_d = nc.dram_tensor("idx_i", [P, idx_cols], I16, kind="ExternalInput").ap()
    ij_d = nc.dram_tensor("idx_j", [P, idx_cols], I16, kind="ExternalInput").ap()
    w_d = nc.dram_tensor("w_bf", [IN_SIZE, EMB_EDGE], BF16, kind="ExternalInput").ap()
    out_d = nc.dram_tensor(
        "out", [e_pad, EMB_EDGE], out_dtype, kind="ExternalOutput"
    ).ap()

    with tile.TileContext(nc) as tc:
        with (
            tc.tile_pool(name="const", bufs=1) as const_pool,
            tc.tile_pool(name="mt", bufs=4) as mt_pool,
            tc.tile_pool(name="hg", bufs=6) as hg_pool,
            tc.tile_pool(name="acc", bufs=8, space="PSUM") as acc_pool,
            tc.tile_pool(name="outp", bufs=6) as out_pool,
        ):
            idxi_t = const_pool.tile([P, idx_cols], I16, tag="idxi")
            nc.sync.dma_start(idxi_t[:], ii_d[:])
            idxj_t = const_pool.tile([P, idx_cols], I16, tag="idxj")
            nc.sync.dma_start(idxj_t[:], ij_d[:])
            w_tile = const_pool.tile([P, K_CHUNKS, EMB_EDGE], BF16)
            nc.scalar.dma_start(w_tile[:], w_d.rearrange("(k p) o -> p k o", p=P))

            mt_r = mt_d.rearrange("(c p) e -> p c e", p=P)  # [128, 4, e_pad]

            tile_base = 0      # global tile counter
            q = 0              # SWDGE queue round-robin

            for g, (i_in_b, j_in_b) in enumerate(GROUPS):
                tabs = {"i": hb_d if i_in_b else ha_d, "j": hb_d if j_in_b else ha_d}
                gt = group_tiles[g]
                for s0 in range(0, gt, tiles_per_slab):
                    nt = min(tiles_per_slab, gt - s0)
                    t0 = tile_base + s0
                    e0 = t0 * P
                    es = nt * P
                    mt_slab = mt_pool.tile([P, M_CHUNKS, es], BF16, tag="mt")
                    nc.sync.dma_start(mt_slab[:], mt_r[:, :, e0 : e0 + es])

                    gat = {}
                    for side in ("i", "j"):
                        idx_t = idxi_t if side == "i" else idxj_t
                        gg = hg_pool.tile([P, H_CHUNKS, es], BF16, tag=f"g{side}")
                        nc.gpsimd.dma_gather(
                            out_ap=gg[:],
                            in_ap=tabs[side],
                            idxs_ap=idx_t[:, e0 // 16 : (e0 + es) // 16],
                            num_idxs=es,
                            num_idxs_reg=es,
                            elem_size=EMB_ATOM,
                            transpose=True,
                            queue_num=q % NUM_SWDGE_QUEUES,
                        )
                        q += 1
                        gat[side] = gg

                    for t in range(nt):
                        acc = acc_pool.tile([P, EMB_EDGE], F32)
                        esl = slice(t * P, (t + 1) * P)
                        for c in range(M_CHUNKS):
                            nc.tensor.matmul(
                                acc[:],
                                lhsT=mt_slab[:, c, esl],
                                rhs=w_tile[:, 2 * H_CHUNKS + c, :],
                                start=(c == 0),
                                stop=False,
                            )
                        for c in range(H_CHUNKS):
                            nc.tensor.matmul(
                                acc[:],
                                lhsT=gat["i"][:, c, esl],
                                rhs=w_tile[:, c, :],
                                start=False,
                                stop=False,
                            )
                        for c in range(H_CHUNKS):
                            nc.tensor.matmul(
                                acc[:],
                                lhsT=gat["j"][:, c, esl],
                                rhs=w_tile[:, H_CHUNKS + c, :],
                                start=False,
                                stop=(c == H_CHUNKS - 1),
                            )

                        ot = out_pool.tile([P, EMB_EDGE], out_dtype)
                        nc.scalar.activation(ot[:], acc[:], act)
                        e_t = e0 + t * P
                        nc.sync.dma_start(out_d[e_t : e_t + P, :], ot[:])
                tile_base += gt
    nc.compile()
    return nc


def _wrap_idx16(vals):
    """[n] int array (n % 128 == 0) -> [128, n//16] int16 in dma_gather
    layout: list element k sits at partition k%16, column k//16, replicated
    across the 8 groups of 16 partitions."""
    n = vals.size
    blk = vals.reshape(n // 16, 16).T.astype(np.int16)  # [16, n/16]
    return np.ascontiguousarray(np.tile(blk, (8, 1)))


def partition_core(ix_i, ix_j, split):
    """Group edges by (table_i, table_j). Returns (order, counts):
    order = edge indices sorted by group (stable), counts per group."""
    gid = (ix_i >= split).astype(np.int8) * 2 + (ix_j >= split)
    order = np.argsort(gid, kind="stable")
    counts = np.bincount(gid, minlength=4)
    return order, counts


def prep_core_inputs(h_a, h_b, w_bf, m, ix_i, ix_j, order, counts, group_tiles,
                     split):
    """Build one core's padded, grouped input map."""
    total_tiles = sum(group_tiles)
    e_pad = total_tiles * P
    m_pad = np.zeros((e_pad, EMB_EDGE), np.float32)
    ii_pad = np.zeros(e_pad, np.int64)
    jj_pad = np.zeros(e_pad, np.int64)
    pos = 0
    off = 0
    for g in range(4):
        n = int(counts[g])
        sel = order[pos : pos + n]
        m_pad[off : off + n] = m[sel]
        ii_pad[off : off + n] = ix_i[sel]
        jj_pad[off : off + n] = ix_j[sel]
        # padding rows keep idx 0, which is valid for either table
        pos += n
        off += group_tiles[g] * P
    i_in_b = np.repeat([b for b, _ in GROUPS], np.array(group_tiles) * P)
    j_in_b = np.repeat([b for _, b in GROUPS], np.array(group_tiles) * P)
    ii_dev = np.where(i_in_b, np.maximum(ii_pad - split, 0), ii_pad)
    jj_dev = np.where(j_in_b, np.maximum(jj_pad - split, 0), jj_pad)
    return {
        "h_a": h_a,
        "h_b": h_b,
        "m_t": np.ascontiguousarray(m_pad.T).astype(ml_dtypes.bfloat16),
        "idx_i": _wrap_idx16(ii_dev),
        "idx_j": _wrap_idx16(jj_dev),
        "w_bf": w_bf,
    }


def _ensure_ntff_hook():
    """Make trace=True work: register the ctypes NTFF profile hook when the
    image's antenv package lacks axon_hooks (boot degrades silently)."""
    import sys
    import types

    try:
        from antenv.axon_hooks import get_axon_ntff_profile_hook  # noqa: F401

        return
    except ImportError:
        pass
    import antenv
    from trn_agent_boot.trn_boot import _ntff_profile_via_ctypes

    hook = _ntff_profile_via_ctypes("/opt/axon/libaxon_pjrt.so")
    mod = types.ModuleType("antenv.axon_hooks")
    mod.get_axon_ntff_profile_hook = lambda: hook
    mod.set_axon_ntff_profile_hook = lambda h: None
    sys.modules["antenv.axon_hooks"] = mod
    antenv.axon_hooks = mod


_NC_CACHE = {}


def kernel(h, m_ij, idx_i, idx_j, W, trace=False, split=SPLIT):
    e_total = m_ij.shape[0]
    e_core = e_total // N_CORES
    if trace:
        _ensure_ntff_hook()

    h_bf = np.asarray(h).astype(ml_dtypes.bfloat16)
    h_a, h_b = h_bf[:split], h_bf[split:]
    if h_b.size == 0:
        h_b = np.zeros((1, EMB_ATOM), ml_dtypes.bfloat16)
    w_bf = np.asarray(W).astype(ml_dtypes.bfloat16)
    idx_i = np.asarray(idx_i)
    idx_j = np.asarray(idx_j)

    parts = []
    for c in range(N_CORES):
        sl = slice(c * e_core, (c + 1) * e_core)
        parts.append(partition_core(idx_i[sl], idx_j[sl], split))
    group_tiles = tuple(
        int(max((p[1][g] + P - 1) // P for p in parts)) for g in range(4)
    )

    key = (group_tiles, split, h.shape[0])
    if key not in _NC_CACHE:
        _NC_CACHE[key] = build_nc(group_tiles, n_nodes=h.shape[0], split=split)
    nc = _NC_CACHE[key]

    in_maps = []
    for c in range(N_CORES):
        sl = slice(c * e_core, (c + 1) * e_core)
        order, counts = parts[c]
        in_maps.append(
            prep_core_inputs(
                h_a, h_b, w_bf, m_ij[sl], idx_i[sl], idx_j[sl],
                order, counts, group_tiles, split,
            )
        )

    res = run_bass_kernel_spmd(nc, in_maps, core_ids=list(range(N_CORES)), trace=trace)

    out = np.empty((e_total, EMB_EDGE), np.float32)
    for c in range(N_CORES):
        order, counts = parts[c]
        dev = res.results[c]["out"]
        pos = 0
        off = 0
        core_out = out[c * e_core : (c + 1) * e_core]
        for g in range(4):
            n = int(counts[g])
            core_out[order[pos : pos + n]] = dev[off : off + n].astype(np.float32)
            pos += n
            off += group_tiles[g] * P
    if trace:
        kernel.last_result = res
    return out
